# revision 11
# baseline (speedup 1.0000x reference)
"""Dense MLP y = x @ W.T + b on 8 TRN2 NeuronCores, data-parallel over batch.

Full inputs: x [8192, 1024] f32, W [1024, 1024] f32, b [1024] f32.
Each core computes a [1024, 1024] slice of the output.

Per-core kernel computes the transposed output
    outT[n, m] = sum_k WT[k, n] * xT[k, m] + b[n]
so the bias lands on the partition dim (n) and fuses into the PSUM
eviction as a DVE tensor_scalar add. Host pre-transposes x-shards and W
to K-major (contraction on partitions) and un-transposes the gathered
outputs; only device time counts.

v4 (trace-driven, fp16): the warm PE streams 1 row/cycle for fp32r,
fp16 and bf16 alike (227 ns per 512-row matmul measured), so the
128-matmul PE floor is ~29 us and fp32 DMA (8 MB loads at ~270 GB/s on
one HWDGE ring) was the binding constraint. Switch x/W/out to fp16
(max-rel-err ~7e-4, gate is 2e-2):
  - loads drop to 4 MB, split across BOTH HWDGE rings (sync ring: W,
    scalar ring: x) which round-robin fairly at packet granularity, so
    w_k/x_k tiles land in lockstep well ahead of the PE;
  - stores drop to 2 MB and alternate rings by group parity.
Tensor program: ~0.9 us of tiny dummy matmuls at t=0 (on a memset
tile) trip the HAM activity window early so the PE reaches 2.4 GHz
sooner; phase A (mb=0) runs k-outer so each arriving k-slice feeds 8
matmuls; phase B (mb=1) runs k-inner per group so group completions
pace 1.8 us apart and evictions+stores pipeline instead of bunching.
The last group is evicted/stored in halves on both rings to shorten
the tail. Raw Bass (no TileContext: its exit drain trips "Too many
sync wait commands" in this compiler build).
"""

import numpy as np

import concourse.bass as bass
import concourse.mybir as mybir
from concourse.bass_utils import run_bass_kernel_spmd

B, IN_F, OUT_F = 8192, 1024, 1024
N_CORES = 8
M = B // N_CORES  # batch rows per core (1024)
P = 128           # partitions
MB = 512          # moving-dim block (one PSUM bank of fp32)
KT = IN_F // P    # k tiles (8)
NT = OUT_F // P   # n tiles (8)
NGROUPS = 16      # (mb, nt) output groups of [128, 512]

F16 = mybir.dt.float16
F32 = mybir.dt.float32


def build_program() -> bass.Bass:
    nc = bass.Bass()
    xT = nc.declare_dram_parameter("xT", [IN_F, M], F16, isOutput=False)
    wT = nc.declare_dram_parameter("wT", [IN_F, OUT_F], F16, isOutput=False)
    bias = nc.declare_dram_parameter("bias", [P, NT], F32, isOutput=False)
    outT = nc.declare_dram_parameter("outT", [OUT_F, M], F16, isOutput=True)

    import contextlib

    with contextlib.ExitStack() as ctx:
        wt_sb = [
            ctx.enter_context(nc.sbuf_tensor(f"wt{k}", [P, OUT_F], F16))
            for k in range(KT)
        ]
        xt_sb = [
            ctx.enter_context(nc.sbuf_tensor(f"xt{k}", [P, M], F16))
            for k in range(KT)
        ]
        ot_sb = [
            ctx.enter_context(nc.sbuf_tensor(f"ot{j}", [P, MB], F16))
            for j in range(8)
        ]
        bias_sb = ctx.enter_context(nc.sbuf_tensor("bias_sb", [P, NT], F32))
        dummy_sb = ctx.enter_context(nc.sbuf_tensor("dummy_sb", [P, P], F16))
        ps = [
            ctx.enter_context(nc.psum_tensor(f"ps{b}", [P, MB], F32))
            for b in range(8)
        ]
        ld_b = ctx.enter_context(nc.semaphore("ld_b"))
        dm = ctx.enter_context(nc.semaphore("dm"))
        # Per-tile load sems: a shared counter can't prove a *specific*
        # DMA finished (completions are unordered), a single-incrementer
        # sem can. k=0 tiles are split small (64 KB quarters) so the
        # first real matmul can start ~1 us earlier.
        ld_w0q = ctx.enter_context(nc.semaphore("ld_w0q"))
        ld_w0r = ctx.enter_context(nc.semaphore("ld_w0r"))
        ld_x0q = [ctx.enter_context(nc.semaphore(f"ld_x0q{c}")) for c in range(2)]
        ld_x0b = ctx.enter_context(nc.semaphore("ld_x0b"))
        ld_w = [ctx.enter_context(nc.semaphore(f"ld_w{k}")) for k in range(1, KT)]
        ld_x = [ctx.enter_context(nc.semaphore(f"ld_x{k}")) for k in range(1, KT)]
        mm = ctx.enter_context(nc.semaphore("mm"))
        ev = ctx.enter_context(nc.semaphore("ev"))
        ev_h = ctx.enter_context(nc.semaphore("ev_h"))  # last-group halves
        st_sems = [ctx.enter_context(nc.semaphore(f"st{j}")) for j in range(8)]
        st_h = ctx.enter_context(nc.semaphore("st_h"))

        def store_ap(g):
            mb, nt = divmod(g, NT)
            return outT[nt * P:(nt + 1) * P, mb * MB:(mb + 1) * MB]

        with nc.Block(no_gpsimd_drain=True) as block:

            @block.sync
            def _(sync):
                # W loads on the sync HWDGE ring, in k order.
                sync.dma_start(
                    out=wt_sb[0][:, 0:2 * P], in_=wT[0:P, 0:2 * P],
                ).then_inc(ld_w0q, 16)
                sync.dma_start(
                    out=wt_sb[0][:, 2 * P:OUT_F], in_=wT[0:P, 2 * P:OUT_F],
                ).then_inc(ld_w0r, 16)
                for k in range(1, KT):
                    sync.dma_start(
                        out=wt_sb[k][:], in_=wT[k * P:(k + 1) * P, :],
                    ).then_inc(ld_w[k - 1], 16)
                # Even-group stores on this ring.
                for g in range(0, NGROUPS - 1, 2):
                    sync.wait_ge(ev, g + 1)
                    sync.dma_start(
                        out=store_ap(g), in_=ot_sb[g % 8][:],
                    ).then_inc(st_sems[g % 8], 16)
                # First half of the final group.
                sync.wait_ge(ev_h, 1)
                sync.dma_start(
                    out=outT[7 * P:8 * P, MB:MB + MB // 2],
                    in_=ot_sb[7][:, 0:MB // 2],
                ).then_inc(st_h, 16)
                for j in range(0, 8, 2):
                    sync.wait_ge(st_sems[j], 32)
                sync.wait_ge(st_h, 32)

            @block.scalar
            def _(scalar):
                # x loads on the scalar HWDGE ring, in k order.
                scalar.dma_start(
                    out=xt_sb[0][:, 0:2 * P], in_=xT[0:P, 0:2 * P],
                ).then_inc(ld_x0q[0], 16)
                scalar.dma_start(
                    out=xt_sb[0][:, 2 * P:MB], in_=xT[0:P, 2 * P:MB],
                ).then_inc(ld_x0q[1], 16)
                scalar.dma_start(
                    out=xt_sb[0][:, MB:M], in_=xT[0:P, MB:M],
                ).then_inc(ld_x0b, 16)
                for k in range(1, KT):
                    scalar.dma_start(
                        out=xt_sb[k][:], in_=xT[k * P:(k + 1) * P, :],
                    ).then_inc(ld_x[k - 1], 16)
                # Odd-group stores on this ring.
                for g in range(1, NGROUPS - 1, 2):
                    scalar.wait_ge(ev, g + 1)
                    scalar.dma_start(
                        out=store_ap(g), in_=ot_sb[g % 8][:],
                    ).then_inc(st_sems[g % 8], 16)
                # Second half of the final group.
                scalar.wait_ge(ev_h, 2)
                scalar.dma_start(
                    out=outT[7 * P:8 * P, MB + MB // 2:2 * MB],
                    in_=ot_sb[7][:, MB // 2:MB],
                ).then_inc(st_h, 16)
                for j in range(1, 8, 2):
                    scalar.wait_ge(st_sems[j], 32 if j != 7 else 16)
                scalar.wait_ge(st_h, 32)

            @block.gpsimd
            def _(gpsimd):
                gpsimd.dma_start(out=bias_sb[:], in_=bias[:]).then_inc(ld_b, 16)

            @block.tensor
            def _(tensor):
                # ~2 us of tiny matmuls on a zeroed tile so the HAM
                # activity window starts counting at t=0 and the PE is
                # at 2.4 GHz when real data lands. Bank 0's junk is
                # overwritten by the first real start=True matmul.
                tensor.wait_ge(dm, 1)
                for _ in range(18):
                    tensor.matmul(
                        ps[0][:, 0:P], dummy_sb[:, 0:P], dummy_sb[:, 0:P],
                        start=True, stop=True,
                    )
                # Phase A (mb=0): k=0 runs off the small quarter tiles
                # the moment they land (the q1 matmuls use start=False:
                # q0's start=True already cleared the whole bank's
                # has_written bits, so q1 still overwrites its region);
                # then k-outer over all 8 banks - each k-slice feeds 8
                # matmuls as it lands.
                tensor.wait_ge(ld_w0q, 16)
                tensor.wait_ge(ld_x0q[0], 16)
                for nt in range(2):
                    tensor.matmul(
                        ps[nt][:, 0:2 * P],
                        wt_sb[0][:, nt * P:(nt + 1) * P],
                        xt_sb[0][:, 0:2 * P],
                        start=True, stop=False,
                    )
                tensor.wait_ge(ld_x0q[1], 16)
                for nt in range(2):
                    tensor.matmul(
                        ps[nt][:, 2 * P:MB],
                        wt_sb[0][:, nt * P:(nt + 1) * P],
                        xt_sb[0][:, 2 * P:MB],
                        start=False, stop=False,
                    )
                tensor.wait_ge(ld_w0r, 16)
                for nt in range(2, NT):
                    tensor.matmul(
                        ps[nt][:, :],
                        wt_sb[0][:, nt * P:(nt + 1) * P],
                        xt_sb[0][:, 0:MB],
                        start=True, stop=False,
                    )
                for k in range(1, KT):
                    tensor.wait_ge(ld_w[k - 1], 16)
                    tensor.wait_ge(ld_x[k - 1], 16)
                    for nt in range(NT):
                        inst = tensor.matmul(
                            ps[nt][:, :],
                            wt_sb[k][:, nt * P:(nt + 1) * P],
                            xt_sb[k][:, 0:MB],
                            start=False,
                            stop=(k == KT - 1),
                        )
                        if k == KT - 1:
                            inst.then_inc(mm, 1)
                # Phase B (mb=1): k-inner per group - completions land
                # ~1.8 us apart so evictions + stores pipeline.
                tensor.wait_ge(ld_x0b, 16)
                for nt in range(NT):
                    tensor.wait_ge(ev, nt + 1)  # bank nt evicted (A)
                    inst = None
                    for k in range(KT):
                        inst = tensor.matmul(
                            ps[nt][:, :],
                            wt_sb[k][:, nt * P:(nt + 1) * P],
                            xt_sb[k][:, MB:2 * MB],
                            start=(k == 0),
                            stop=(k == KT - 1),
                        )
                    inst.then_inc(mm, 1)

            @block.vector
            def _(vector):
                vector.memset(dummy_sb[:], 0.0).then_inc(dm, 1)
                vector.wait_ge(ld_b, 16)
                for g in range(NGROUPS - 1):
                    mb, nt = divmod(g, NT)
                    vector.wait_ge(mm, g + 1)
                    if g >= 8:
                        # ot slot g-8 reused: its store must be done
                        vector.wait_ge(st_sems[g - 8], 16)
                    vector.tensor_scalar_add(
                        ot_sb[g % 8][:],
                        ps[g % 8][:, :],
                        bias_sb[:, nt:nt + 1],
                    ).then_inc(ev, 1)
                # Last group in halves: first half's store overlaps the
                # second half's eviction, shortening the critical tail.
                vector.wait_ge(mm, NGROUPS)
                vector.wait_ge(st_sems[7], 16)
                for h in range(2):
                    vector.tensor_scalar_add(
                        ot_sb[7][:, h * (MB // 2):(h + 1) * (MB // 2)],
                        ps[7][:, h * (MB // 2):(h + 1) * (MB // 2)],
                        bias_sb[:, 7:8],
                    ).then_inc(ev_h, 1)

    return nc


_PROGRAM = None


def _get_program() -> bass.Bass:
    global _PROGRAM
    if _PROGRAM is None:
        _PROGRAM = build_program()
    return _PROGRAM


def make_in_maps(x: np.ndarray, W: np.ndarray, b: np.ndarray) -> list[dict]:
    WT = np.ascontiguousarray(W.T.astype(np.float16))
    bias = np.ascontiguousarray(
        b.astype(np.float32, copy=False).reshape(NT, P).T
    )
    in_maps = []
    for c in range(N_CORES):
        xT = np.ascontiguousarray(x[c * M:(c + 1) * M, :].T.astype(np.float16))
        in_maps.append({"xT": xT, "wT": WT, "bias": bias})
    return in_maps


def assemble_output(results: list[dict]) -> np.ndarray:
    out = np.empty((B, OUT_F), dtype=np.float32)
    for c in range(N_CORES):
        out[c * M:(c + 1) * M, :] = results[c]["outT"].T.astype(np.float32)
    return out


def kernel(x: np.ndarray, W: np.ndarray, b: np.ndarray) -> np.ndarray:
    nc = _get_program()
    in_maps = make_in_maps(np.asarray(x), np.asarray(W), np.asarray(b))
    res = run_bass_kernel_spmd(nc, in_maps, list(range(N_CORES)))
    return assemble_output(res.results)


# revision 15
# speedup vs baseline: 1.0137x; 1.0137x over previous
"""Dense MLP y = x @ W.T + b on 8 TRN2 NeuronCores, data-parallel over batch.

Full inputs: x [8192, 1024] f32, W [1024, 1024] f32, b [1024] f32.
Each core computes a [1024, 1024] slice of the output.

Per-core kernel computes the transposed output
    outT[n, m] = sum_k WT[k, n] * xT[k, m] + b[n]
so the bias lands on the partition dim (n) and fuses into the PSUM
eviction as a DVE tensor_scalar add. Host pre-transposes x-shards and W
to K-major (contraction on partitions) and un-transposes the gathered
outputs; only device time counts.

v4 (trace-driven, fp16): the warm PE streams 1 row/cycle for fp32r,
fp16 and bf16 alike (227 ns per 512-row matmul measured), so the
128-matmul PE floor is ~29 us and fp32 DMA (8 MB loads at ~270 GB/s on
one HWDGE ring) was the binding constraint. Switch x/W/out to fp16
(max-rel-err ~7e-4, gate is 2e-2):
  - loads drop to 4 MB, split across BOTH HWDGE rings (sync ring: W,
    scalar ring: x) which round-robin fairly at packet granularity, so
    w_k/x_k tiles land in lockstep well ahead of the PE;
  - stores drop to 2 MB and alternate rings by group parity.
Tensor program: ~0.9 us of tiny dummy matmuls at t=0 (on a memset
tile) trip the HAM activity window early so the PE reaches 2.4 GHz
sooner; phase A (mb=0) runs k-outer so each arriving k-slice feeds 8
matmuls; phase B (mb=1) runs k-inner per group so group completions
pace 1.8 us apart and evictions+stores pipeline instead of bunching.
The last group is evicted/stored in halves on both rings to shorten
the tail. Raw Bass (no TileContext: its exit drain trips "Too many
sync wait commands" in this compiler build).
"""

import numpy as np

import concourse.bass as bass
import concourse.mybir as mybir
from concourse.bass_utils import run_bass_kernel_spmd

B, IN_F, OUT_F = 8192, 1024, 1024
N_CORES = 8
M = B // N_CORES  # batch rows per core (1024)
P = 128           # partitions
MB = 512          # moving-dim block (one PSUM bank of fp32)
KT = IN_F // P    # k tiles (8)
NT = OUT_F // P   # n tiles (8)
NGROUPS = 16      # (mb, nt) output groups of [128, 512]

F16 = mybir.dt.float16
F32 = mybir.dt.float32


def build_program() -> bass.Bass:
    nc = bass.Bass()
    xT = nc.declare_dram_parameter("xT", [IN_F, M], F16, isOutput=False)
    wT = nc.declare_dram_parameter("wT", [IN_F, OUT_F], F16, isOutput=False)
    bias = nc.declare_dram_parameter("bias", [P, NT], F32, isOutput=False)
    outT = nc.declare_dram_parameter("outT", [OUT_F, M], F16, isOutput=True)

    import contextlib

    with contextlib.ExitStack() as ctx:
        wt_sb = [
            ctx.enter_context(nc.sbuf_tensor(f"wt{k}", [P, OUT_F], F16))
            for k in range(KT)
        ]
        xt_sb = [
            ctx.enter_context(nc.sbuf_tensor(f"xt{k}", [P, M], F16))
            for k in range(KT)
        ]
        ot_sb = [
            ctx.enter_context(nc.sbuf_tensor(f"ot{j}", [P, MB], F16))
            for j in range(8)
        ]
        bias_sb = ctx.enter_context(nc.sbuf_tensor("bias_sb", [P, NT], F32))
        dummy_sb = ctx.enter_context(nc.sbuf_tensor("dummy_sb", [P, P], F16))
        ps = [
            ctx.enter_context(nc.psum_tensor(f"ps{b}", [P, MB], F32))
            for b in range(8)
        ]
        ld_b = ctx.enter_context(nc.semaphore("ld_b"))
        dm = ctx.enter_context(nc.semaphore("dm"))
        # Per-tile load sems: a shared counter can't prove a *specific*
        # DMA finished (completions are unordered), a single-incrementer
        # sem can. k=0 tiles are split small (64 KB quarters) so the
        # first real matmul can start ~1 us earlier.
        ld_w0a = ctx.enter_context(nc.semaphore("ld_w0a"))
        ld_w0b = ctx.enter_context(nc.semaphore("ld_w0b"))
        ld_x0a = ctx.enter_context(nc.semaphore("ld_x0a"))
        ld_x0b = ctx.enter_context(nc.semaphore("ld_x0b"))
        ld_w = [ctx.enter_context(nc.semaphore(f"ld_w{k}")) for k in range(1, KT)]
        ld_x = [ctx.enter_context(nc.semaphore(f"ld_x{k}")) for k in range(1, KT)]
        mm = ctx.enter_context(nc.semaphore("mm"))
        ev = ctx.enter_context(nc.semaphore("ev"))
        ev_h = ctx.enter_context(nc.semaphore("ev_h"))  # last-group halves
        st_sems = [ctx.enter_context(nc.semaphore(f"st{j}")) for j in range(8)]
        st_h = ctx.enter_context(nc.semaphore("st_h"))

        def store_ap(g):
            mb, nt = divmod(g, NT)
            return outT[nt * P:(nt + 1) * P, mb * MB:(mb + 1) * MB]

        with nc.Block(no_gpsimd_drain=True) as block:

            @block.sync
            def _(sync):
                # W loads on the sync HWDGE ring, in k order.
                sync.dma_start(
                    out=wt_sb[0][:, 0:MB], in_=wT[0:P, 0:MB],
                ).then_inc(ld_w0a, 16)
                sync.dma_start(
                    out=wt_sb[0][:, MB:OUT_F], in_=wT[0:P, MB:OUT_F],
                ).then_inc(ld_w0b, 16)
                for k in range(1, KT):
                    sync.dma_start(
                        out=wt_sb[k][:], in_=wT[k * P:(k + 1) * P, :],
                    ).then_inc(ld_w[k - 1], 16)
                # Even-group stores on this ring.
                for g in range(0, NGROUPS - 1, 2):
                    sync.wait_ge(ev, g + 1)
                    sync.dma_start(
                        out=store_ap(g), in_=ot_sb[g % 8][:],
                    ).then_inc(st_sems[g % 8], 16)
                # First half of the final group.
                sync.wait_ge(ev_h, 1)
                sync.dma_start(
                    out=outT[7 * P:8 * P, MB:MB + MB // 2],
                    in_=ot_sb[7][:, 0:MB // 2],
                ).then_inc(st_h, 16)
                for j in range(0, 8, 2):
                    sync.wait_ge(st_sems[j], 32)
                sync.wait_ge(st_h, 32)

            @block.scalar
            def _(scalar):
                # x loads on the scalar HWDGE ring, in k order.
                scalar.dma_start(
                    out=xt_sb[0][:, 0:MB], in_=xT[0:P, 0:MB],
                ).then_inc(ld_x0a, 16)
                scalar.dma_start(
                    out=xt_sb[0][:, MB:M], in_=xT[0:P, MB:M],
                ).then_inc(ld_x0b, 16)
                for k in range(1, KT):
                    scalar.dma_start(
                        out=xt_sb[k][:], in_=xT[k * P:(k + 1) * P, :],
                    ).then_inc(ld_x[k - 1], 16)
                # Odd-group stores on this ring.
                for g in range(1, NGROUPS - 1, 2):
                    scalar.wait_ge(ev, g + 1)
                    scalar.dma_start(
                        out=store_ap(g), in_=ot_sb[g % 8][:],
                    ).then_inc(st_sems[g % 8], 16)
                # Second half of the final group.
                scalar.wait_ge(ev_h, 2)
                scalar.dma_start(
                    out=outT[7 * P:8 * P, MB + MB // 2:2 * MB],
                    in_=ot_sb[7][:, MB // 2:MB],
                ).then_inc(st_h, 16)
                for j in range(1, 8, 2):
                    scalar.wait_ge(st_sems[j], 32 if j != 7 else 16)
                scalar.wait_ge(st_h, 32)

            @block.gpsimd
            def _(gpsimd):
                gpsimd.dma_start(out=bias_sb[:], in_=bias[:]).then_inc(ld_b, 16)

            @block.tensor
            def _(tensor):
                # ~3.2 us of tiny matmuls on a zeroed tile: the HAM
                # activity window needs ~3.4 us of GAPLESS PE busy-ness
                # before it lifts the clock to 2.4 GHz, and any pre-warm
                # idle gap restarts the count (post-warm sub-us gaps are
                # harmless - re-throttle needs a ~3.4 us idle window).
                # The bridge spans until the first real tiles are
                # reliably resident, so the real stream starts warm.
                # Bank 0's junk is overwritten by the first real
                # start=True matmul.
                tensor.wait_ge(dm, 1)
                for _ in range(30):
                    tensor.matmul(
                        ps[0][:, 0:P], dummy_sb[:, 0:P], dummy_sb[:, 0:P],
                        start=True, stop=True,
                    )
                # Phase A (mb=0): k-outer over all 8 banks - each
                # k-slice feeds 8 matmuls the moment it lands.
                tensor.wait_ge(ld_w0a, 16)
                tensor.wait_ge(ld_x0a, 16)
                for nt in range(NT):
                    if nt == 4:
                        tensor.wait_ge(ld_w0b, 16)
                    tensor.matmul(
                        ps[nt][:, :],
                        wt_sb[0][:, nt * P:(nt + 1) * P],
                        xt_sb[0][:, 0:MB],
                        start=True, stop=False,
                    )
                for k in range(1, KT):
                    tensor.wait_ge(ld_w[k - 1], 16)
                    tensor.wait_ge(ld_x[k - 1], 16)
                    for nt in range(NT):
                        inst = tensor.matmul(
                            ps[nt][:, :],
                            wt_sb[k][:, nt * P:(nt + 1) * P],
                            xt_sb[k][:, 0:MB],
                            start=False,
                            stop=(k == KT - 1),
                        )
                        if k == KT - 1:
                            inst.then_inc(mm, 1)
                # Phase B (mb=1): k-inner per group - completions land
                # ~1.8 us apart so evictions + stores pipeline.
                tensor.wait_ge(ld_x0b, 16)
                for nt in range(NT):
                    tensor.wait_ge(ev, nt + 1)  # bank nt evicted (A)
                    inst = None
                    for k in range(KT):
                        inst = tensor.matmul(
                            ps[nt][:, :],
                            wt_sb[k][:, nt * P:(nt + 1) * P],
                            xt_sb[k][:, MB:2 * MB],
                            start=(k == 0),
                            stop=(k == KT - 1),
                        )
                    inst.then_inc(mm, 1)

            @block.vector
            def _(vector):
                vector.memset(dummy_sb[:], 0.0).then_inc(dm, 1)
                vector.wait_ge(ld_b, 16)
                for g in range(NGROUPS - 1):
                    mb, nt = divmod(g, NT)
                    vector.wait_ge(mm, g + 1)
                    if g >= 8:
                        # ot slot g-8 reused: its store must be done
                        vector.wait_ge(st_sems[g - 8], 16)
                    vector.tensor_scalar_add(
                        ot_sb[g % 8][:],
                        ps[g % 8][:, :],
                        bias_sb[:, nt:nt + 1],
                    ).then_inc(ev, 1)
                # Last group in halves: first half's store overlaps the
                # second half's eviction, shortening the critical tail.
                vector.wait_ge(mm, NGROUPS)
                vector.wait_ge(st_sems[7], 16)
                for h in range(2):
                    vector.tensor_scalar_add(
                        ot_sb[7][:, h * (MB // 2):(h + 1) * (MB // 2)],
                        ps[7][:, h * (MB // 2):(h + 1) * (MB // 2)],
                        bias_sb[:, 7:8],
                    ).then_inc(ev_h, 1)

    return nc


_PROGRAM = None


def _get_program() -> bass.Bass:
    global _PROGRAM
    if _PROGRAM is None:
        _PROGRAM = build_program()
    return _PROGRAM


def make_in_maps(x: np.ndarray, W: np.ndarray, b: np.ndarray) -> list[dict]:
    WT = np.ascontiguousarray(W.T.astype(np.float16))
    bias = np.ascontiguousarray(
        b.astype(np.float32, copy=False).reshape(NT, P).T
    )
    in_maps = []
    for c in range(N_CORES):
        xT = np.ascontiguousarray(x[c * M:(c + 1) * M, :].T.astype(np.float16))
        in_maps.append({"xT": xT, "wT": WT, "bias": bias})
    return in_maps


def assemble_output(results: list[dict]) -> np.ndarray:
    out = np.empty((B, OUT_F), dtype=np.float32)
    for c in range(N_CORES):
        out[c * M:(c + 1) * M, :] = results[c]["outT"].T.astype(np.float32)
    return out


def kernel(x: np.ndarray, W: np.ndarray, b: np.ndarray) -> np.ndarray:
    nc = _get_program()
    in_maps = make_in_maps(np.asarray(x), np.asarray(W), np.asarray(b))
    res = run_bass_kernel_spmd(nc, in_maps, list(range(N_CORES)))
    return assemble_output(res.results)


# revision 20
# speedup vs baseline: 1.0502x; 1.0360x over previous
"""Dense MLP y = x @ W.T + b on 8 TRN2 NeuronCores, data-parallel over batch.

Full inputs: x [8192, 1024] f32, W [1024, 1024] f32, b [1024] f32.
Each core computes a [1024, 1024] slice of the output.

Per-core kernel computes the transposed output
    outT[n, m] = sum_k WT[k, n] * xT[k, m] + b[n]
so the bias lands on the partition dim (n) and fuses into the PSUM
eviction as a DVE tensor_scalar add. Host pre-transposes x-shards and W
to K-major (contraction on partitions) and un-transposes the gathered
outputs; only device time counts.

v4 (trace-driven, fp16): the warm PE streams 1 row/cycle for fp32r,
fp16 and bf16 alike (227 ns per 512-row matmul measured), so the
128-matmul PE floor is ~29 us and fp32 DMA (8 MB loads at ~270 GB/s on
one HWDGE ring) was the binding constraint. Switch x/W/out to fp16
(max-rel-err ~7e-4, gate is 2e-2):
  - loads drop to 4 MB, split across BOTH HWDGE rings (sync ring: W,
    scalar ring: x) which round-robin fairly at packet granularity, so
    w_k/x_k tiles land in lockstep well ahead of the PE;
  - stores drop to 2 MB and alternate rings by group parity.
Tensor program: ~0.9 us of tiny dummy matmuls at t=0 (on a memset
tile) trip the HAM activity window early so the PE reaches 2.4 GHz
sooner; phase A (mb=0) runs k-outer so each arriving k-slice feeds 8
matmuls; phase B (mb=1) runs k-inner per group so group completions
pace 1.8 us apart and evictions+stores pipeline instead of bunching.
The last group is evicted/stored in halves on both rings to shorten
the tail. Raw Bass (no TileContext: its exit drain trips "Too many
sync wait commands" in this compiler build).
"""

import numpy as np

import concourse.bass as bass
import concourse.mybir as mybir
from concourse.bass_utils import run_bass_kernel_spmd

B, IN_F, OUT_F = 8192, 1024, 1024
N_CORES = 8
M = B // N_CORES  # batch rows per core (1024)
P = 128           # partitions
MB = 512          # moving-dim block (one PSUM bank of fp32)
KT = IN_F // P    # k tiles (8)
NT = OUT_F // P   # n tiles (8)
NGROUPS = 16      # (mb, nt) output groups of [128, 512]

F16 = mybir.dt.float16
F32 = mybir.dt.float32


def build_program() -> bass.Bass:
    nc = bass.Bass()
    xT = nc.declare_dram_parameter("xT", [IN_F, M], F16, isOutput=False)
    wT = nc.declare_dram_parameter("wT", [IN_F, OUT_F], F16, isOutput=False)
    bias = nc.declare_dram_parameter("bias", [P, NT], F32, isOutput=False)
    outT = nc.declare_dram_parameter("outT", [OUT_F, M], F16, isOutput=True)

    import contextlib

    with contextlib.ExitStack() as ctx:
        wt_sb = [
            ctx.enter_context(nc.sbuf_tensor(f"wt{k}", [P, OUT_F], F16))
            for k in range(KT)
        ]
        xt_sb = [
            ctx.enter_context(nc.sbuf_tensor(f"xt{k}", [P, M], F16))
            for k in range(KT)
        ]
        ot_sb = [
            ctx.enter_context(nc.sbuf_tensor(f"ot{j}", [P, MB], F16))
            for j in range(8)
        ]
        bias_sb = ctx.enter_context(nc.sbuf_tensor("bias_sb", [P, NT], F32))
        dummy_sb = ctx.enter_context(nc.sbuf_tensor("dummy_sb", [P, P], F16))
        ps = [
            ctx.enter_context(nc.psum_tensor(f"ps{b}", [P, MB], F32))
            for b in range(8)
        ]
        ld_b = ctx.enter_context(nc.semaphore("ld_b"))
        dm = ctx.enter_context(nc.semaphore("dm"))
        # Per-tile load sems: a shared counter can't prove a *specific*
        # DMA finished (completions are unordered), a single-incrementer
        # sem can. k=0 tiles are split small (64 KB quarters) so the
        # first real matmul can start ~1 us earlier.
        ld_w0a = ctx.enter_context(nc.semaphore("ld_w0a"))
        ld_w0b = ctx.enter_context(nc.semaphore("ld_w0b"))
        ld_w = [ctx.enter_context(nc.semaphore(f"ld_w{k}")) for k in range(1, KT)]
        ld_xa = [ctx.enter_context(nc.semaphore(f"ld_xa{k}")) for k in range(KT)]
        ld_xb = [ctx.enter_context(nc.semaphore(f"ld_xb{k}")) for k in range(KT)]
        mm = ctx.enter_context(nc.semaphore("mm"))
        ev = ctx.enter_context(nc.semaphore("ev"))
        ev_h = ctx.enter_context(nc.semaphore("ev_h"))  # last-group halves
        st_sems = [ctx.enter_context(nc.semaphore(f"st{j}")) for j in range(8)]
        st_h = ctx.enter_context(nc.semaphore("st_h"))

        def store_ap(g):
            mb, nt = divmod(g, NT)
            return outT[nt * P:(nt + 1) * P, mb * MB:(mb + 1) * MB]

        with nc.Block(no_gpsimd_drain=True) as block:

            def load_w(eng, k):
                eng.dma_start(
                    out=wt_sb[k][:], in_=wT[k * P:(k + 1) * P, :],
                ).then_inc(ld_w[k - 1], 16)

            def load_xa(eng, k):
                eng.dma_start(
                    out=xt_sb[k][:, 0:MB], in_=xT[k * P:(k + 1) * P, 0:MB],
                ).then_inc(ld_xa[k], 16)

            def load_xb(eng, k):
                eng.dma_start(
                    out=xt_sb[k][:, MB:M], in_=xT[k * P:(k + 1) * P, MB:M],
                ).then_inc(ld_xb[k], 16)

            # The two HWDGE rings (sync, scalar) round-robin at packet
            # granularity and a DMA's completion sem fires only when the
            # LAST of its 16 SDMA-engine sub-streams is done - under deep
            # queues that last increment lags ~1.5-2.5 us. So W and x
            # slices are interleaved across both rings such that each
            # k-slice's gating DMAs sit early in their ring, and the
            # mb1 x-halves (not needed until phase B at ~24 us) load
            # after all phase-A tiles.
            @block.sync
            def _(sync):
                sync.dma_start(
                    out=wt_sb[0][:, 0:MB], in_=wT[0:P, 0:MB],
                ).then_inc(ld_w0a, 16)
                sync.dma_start(
                    out=wt_sb[0][:, MB:OUT_F], in_=wT[0:P, MB:OUT_F],
                ).then_inc(ld_w0b, 16)
                load_xa(sync, 1)
                load_w(sync, 2)
                load_xa(sync, 3)
                load_w(sync, 4)
                load_xa(sync, 5)
                load_w(sync, 6)
                for k in (0, 2, 4, 6):
                    load_xb(sync, k)
                # Even-group stores on this ring.
                for g in range(0, NGROUPS - 1, 2):
                    sync.wait_ge(ev, g + 1)
                    sync.dma_start(
                        out=store_ap(g), in_=ot_sb[g % 8][:],
                    ).then_inc(st_sems[g % 8], 16)
                # First half of the final group.
                sync.wait_ge(ev_h, 1)
                sync.dma_start(
                    out=outT[7 * P:8 * P, MB:MB + MB // 2],
                    in_=ot_sb[7][:, 0:MB // 2],
                ).then_inc(st_h, 16)
                for j in range(0, 8, 2):
                    sync.wait_ge(st_sems[j], 32)
                sync.wait_ge(st_h, 32)

            @block.scalar
            def _(scalar):
                load_xa(scalar, 0)
                load_w(scalar, 1)
                load_xa(scalar, 2)
                load_w(scalar, 3)
                load_xa(scalar, 4)
                load_w(scalar, 5)
                load_xa(scalar, 6)
                load_w(scalar, 7)
                load_xa(scalar, 7)
                for k in (1, 3, 5, 7):
                    load_xb(scalar, k)
                # Odd-group stores on this ring.
                for g in range(1, NGROUPS - 1, 2):
                    scalar.wait_ge(ev, g + 1)
                    scalar.dma_start(
                        out=store_ap(g), in_=ot_sb[g % 8][:],
                    ).then_inc(st_sems[g % 8], 16)
                # Second half of the final group.
                scalar.wait_ge(ev_h, 2)
                scalar.dma_start(
                    out=outT[7 * P:8 * P, MB + MB // 2:2 * MB],
                    in_=ot_sb[7][:, MB // 2:MB],
                ).then_inc(st_h, 16)
                for j in range(1, 8, 2):
                    scalar.wait_ge(st_sems[j], 32 if j != 7 else 16)
                scalar.wait_ge(st_h, 32)

            @block.gpsimd
            def _(gpsimd):
                gpsimd.dma_start(out=bias_sb[:], in_=bias[:]).then_inc(ld_b, 16)

            @block.tensor
            def _(tensor):
                # ~3.2 us of tiny matmuls on a zeroed tile: the HAM
                # activity window needs ~3.4 us of GAPLESS PE busy-ness
                # before it lifts the clock to 2.4 GHz, and any pre-warm
                # idle gap restarts the count (post-warm sub-us gaps are
                # harmless - re-throttle needs a ~3.4 us idle window).
                # The bridge spans until the first real tiles are
                # reliably resident, so the real stream starts warm.
                # Bank 0's junk is overwritten by the first real
                # start=True matmul.
                tensor.wait_ge(dm, 1)
                for _ in range(26):
                    tensor.matmul(
                        ps[0][:, 0:P], dummy_sb[:, 0:P], dummy_sb[:, 0:P],
                        start=True, stop=True,
                    )
                # Phase A (mb=0): k-outer over all 8 banks - each
                # k-slice feeds 8 matmuls the moment it lands.
                tensor.wait_ge(ld_w0a, 16)
                tensor.wait_ge(ld_xa[0], 16)
                for nt in range(NT):
                    if nt == 4:
                        tensor.wait_ge(ld_w0b, 16)
                    tensor.matmul(
                        ps[nt][:, :],
                        wt_sb[0][:, nt * P:(nt + 1) * P],
                        xt_sb[0][:, 0:MB],
                        start=True, stop=False,
                    )
                for k in range(1, KT):
                    tensor.wait_ge(ld_w[k - 1], 16)
                    tensor.wait_ge(ld_xa[k], 16)
                    for nt in range(NT):
                        inst = tensor.matmul(
                            ps[nt][:, :],
                            wt_sb[k][:, nt * P:(nt + 1) * P],
                            xt_sb[k][:, 0:MB],
                            start=False,
                            stop=(k == KT - 1),
                        )
                        if k == KT - 1:
                            inst.then_inc(mm, 1)
                # Phase B (mb=1): k-inner per group - completions land
                # ~1.8 us apart so evictions + stores pipeline.
                for k in range(KT):
                    tensor.wait_ge(ld_xb[k], 16)
                for nt in range(NT):
                    tensor.wait_ge(ev, nt + 1)  # bank nt evicted (A)
                    inst = None
                    for k in range(KT):
                        inst = tensor.matmul(
                            ps[nt][:, :],
                            wt_sb[k][:, nt * P:(nt + 1) * P],
                            xt_sb[k][:, MB:2 * MB],
                            start=(k == 0),
                            stop=(k == KT - 1),
                        )
                    inst.then_inc(mm, 1)

            @block.vector
            def _(vector):
                vector.memset(dummy_sb[:], 0.0).then_inc(dm, 1)
                vector.wait_ge(ld_b, 16)
                for g in range(NGROUPS - 1):
                    mb, nt = divmod(g, NT)
                    vector.wait_ge(mm, g + 1)
                    if g >= 8:
                        # ot slot g-8 reused: its store must be done
                        vector.wait_ge(st_sems[g - 8], 16)
                    vector.tensor_scalar_add(
                        ot_sb[g % 8][:],
                        ps[g % 8][:, :],
                        bias_sb[:, nt:nt + 1],
                    ).then_inc(ev, 1)
                # Last group in halves: first half's store overlaps the
                # second half's eviction, shortening the critical tail.
                vector.wait_ge(mm, NGROUPS)
                vector.wait_ge(st_sems[7], 16)
                for h in range(2):
                    vector.tensor_scalar_add(
                        ot_sb[7][:, h * (MB // 2):(h + 1) * (MB // 2)],
                        ps[7][:, h * (MB // 2):(h + 1) * (MB // 2)],
                        bias_sb[:, 7:8],
                    ).then_inc(ev_h, 1)

    return nc


_PROGRAM = None


def _get_program() -> bass.Bass:
    global _PROGRAM
    if _PROGRAM is None:
        _PROGRAM = build_program()
    return _PROGRAM


def make_in_maps(x: np.ndarray, W: np.ndarray, b: np.ndarray) -> list[dict]:
    WT = np.ascontiguousarray(W.T.astype(np.float16))
    bias = np.ascontiguousarray(
        b.astype(np.float32, copy=False).reshape(NT, P).T
    )
    in_maps = []
    for c in range(N_CORES):
        xT = np.ascontiguousarray(x[c * M:(c + 1) * M, :].T.astype(np.float16))
        in_maps.append({"xT": xT, "wT": WT, "bias": bias})
    return in_maps


def assemble_output(results: list[dict]) -> np.ndarray:
    out = np.empty((B, OUT_F), dtype=np.float32)
    for c in range(N_CORES):
        out[c * M:(c + 1) * M, :] = results[c]["outT"].T.astype(np.float32)
    return out


def kernel(x: np.ndarray, W: np.ndarray, b: np.ndarray) -> np.ndarray:
    nc = _get_program()
    in_maps = make_in_maps(np.asarray(x), np.asarray(W), np.asarray(b))
    res = run_bass_kernel_spmd(nc, in_maps, list(range(N_CORES)))
    return assemble_output(res.results)
